# revision 22
# baseline (speedup 1.0000x reference)
"""AdaptiveTokenSampling on 8 TRN2 NeuronCores (Bass/Tile, data-parallel over batch).

kernel(**inputs) takes the FULL inputs and returns the FULL outputs
(new_attn f32, new_mask bool, uniq_ids int32), sharding batch B=16 as 2
batches per core. All per-batch work (scoring, CDF, inverse-CDF sampling,
dedup, gather) runs on-device; the host only shards inputs and concatenates
shard outputs.

Numerical strategy: the sampled token ids are discrete argmin decisions, so
the on-device CDF must track the float32 reference bit-closely. All
real-valued arithmetic runs on DVE/ACT in IEEE f32 (sequential prefix scan
for the cumsum); sqrt is computed as v*rsqrt(v) with a table seed polished
by two Newton iterations. The nearest-CDF-index argmin is computed as a
midpoint count — sampled[s]-1 = #{t: cdf[t]+cdf[t+1] < 2*step_s*total'} —
which reproduces jnp.argmin's first-index tie-breaking exactly.
"""

from contextlib import ExitStack

import numpy as np

import concourse.bacc as bacc
import concourse.bass as bass
import concourse.mybir as mybir
import concourse.tile as tile
from concourse.bass_utils import run_bass_kernel_spmd

F32 = mybir.dt.float32
I32 = mybir.dt.int32
I16 = mybir.dt.int16

N_CORES = 8
B = 16
B_LOC = B // N_CORES
H = 16
N = 1025
T = 1024
D = 64
K = 256
S = K - 1
ROWS = B_LOC * H * N

AX = mybir.AxisListType.X
OP = mybir.AluOpType
AF = mybir.ActivationFunctionType


def _emit(tc, ins, outs):
    """Emit one NeuronCore's program (B_LOC batches, all heads) into tc."""
    nc = tc.nc
    x_ap = ins["x"]              # (B_LOC,16,1025,64) f32
    attn_ap = ins["attn"]        # (ROWS,1025) f32
    steps2_ap = ins["steps2c"]   # (128,2) f32: 2*steps[p+128*c] (pad tail)
    out_attn = outs["out_attn"]  # (B_LOC,16,256,1025) f32
    out_ids = outs["out_ids"]    # (B_LOC,256) int32

    attn3 = attn_ap.rearrange("(r n) m -> r n m", n=N)

    with ExitStack() as ctx:
        cp = ctx.enter_context(tc.tile_pool(name="const", bufs=1))
        sb = ctx.enter_context(tc.tile_pool(name="sb", bufs=2))
        xp = ctx.enter_context(tc.tile_pool(name="xp", bufs=2))
        gp = ctx.enter_context(tc.tile_pool(name="gp", bufs=8))
        pp = ctx.enter_context(tc.tile_pool(name="pp", bufs=2, space="PSUM"))


        # phase A: batch 0's x halves get both HWDGE rings to themselves
        # (each ring spans 8 SDMA engines); batch 1's x and the cls rows ride
        # the gpsimd SWDGE ring, which is idle until the first gathers. The
        # sequencers are in-order, so bulk loads go ahead of everything.
        # cls rows first on the gpsimd ring (needed by the sig multiply);
        # then batch 0's x over all three DMA rings (fastest possible head)
        # and batch 1's over the two HWDGE rings only, so the gpsimd ring
        # frees up for batch 0's broadcast + gathers.
        cls_l, xh_l = [], []
        for b in range(B_LOC):
            cls_all = sb.tile([128, H * 8], F32, tag="cls_all")
            nc.gpsimd.dma_start(
                out=cls_all[:].rearrange("p (h j) -> p h j", j=8),
                in_=bass.AP(
                    attn_ap.tensor,
                    b * H * N * N + 1,
                    [[8, 128], [N * N, H], [1, 8]],
                ),
            )
            cls_l.append(cls_all)
        for b in range(B_LOC):
            xq = sb.tile([128, H * 512], F32, tag="xq")
            xh_l.append(xq)
            rings = (nc.sync, nc.scalar, nc.gpsimd) if b == 0 else (
                nc.sync, nc.scalar)
            for pc in range(8):
                rings[pc % len(rings)].dma_start(
                    out=xq[:, pc * 1024:(pc + 1) * 1024],
                    in_=x_ap[b, 2 * pc:2 * pc + 2, 1:, :].rearrange(
                        "h (p j) d -> p h (j d)", p=128
                    ),
                )

        steps2c = cp.tile([128, 2], F32)
        nc.sync.dma_start(out=steps2c[:], in_=steps2_ap[:, :])
        # integer/selection constants for the matmul-based dedup
        km = cp.tile([128, 4 * 128 + 256], F32)
        nc.sync.dma_start(out=km[:], in_=ins["consts"][:, :])
        U_ = km[:, 0:128]          # U[p,j] = p<=j
        AL = km[:, 128:256]        # all-ones
        SH = km[:, 256:384]        # SH[p,j] = (p == j-1)
        E127 = km[:, 384:512]      # [p,j] = (p==127 and j==0)
        IOTA = km[:, 512:768]      # IOTA[p,f] = f

        for b in range(B_LOC):
            cls_all = cls_l[b]
            xq = xh_l[b]

            # squared value norms: ACT squares + DVE grouped-reduces,
            # pipelined at 2-head granularity behind the split loads
            vnsq = sb.tile([128, H * 8], F32, tag="vnsq")
            for pc in range(8):
                sq = xp.tile([128, 2 * 512], F32, tag="sq")
                nc.scalar.activation(
                    out=sq[:], in_=xq[:, pc * 1024:(pc + 1) * 1024],
                    func=AF.Square,
                )
                nc.vector.tensor_reduce(
                    out=vnsq[:, pc * 16:(pc + 1) * 16],
                    in_=sq[:].rearrange("p (hj d) -> p hj d", d=D),
                    axis=AX,
                    op=OP.add,
                )

            # vn = vnsq * rsqrt(vnsq); seed 1/ACT-sqrt + 2 Newton iterations
            r = sb.tile([128, H * 8], F32, tag="r")
            nc.scalar.activation(out=r[:], in_=vnsq[:], func=AF.Sqrt)
            nc.vector.reciprocal(out=r[:], in_=r[:])
            t1 = sb.tile([128, H * 8], F32, tag="t1")
            for _ in range(2):
                nc.vector.tensor_mul(out=t1[:], in0=r[:], in1=r[:])
                nc.vector.tensor_mul(out=t1[:], in0=t1[:], in1=vnsq[:])
                nc.vector.tensor_scalar(
                    out=t1[:], in0=t1[:], scalar1=-0.5, scalar2=1.5,
                    op0=OP.mult, op1=OP.add,
                )
                nc.vector.tensor_mul(out=r[:], in0=r[:], in1=t1[:])
            vn = sb.tile([128, H * 8], F32, tag="vn")
            nc.vector.tensor_mul(out=vn[:], in0=vnsq[:], in1=r[:])

            # head-summed significance
            prod = sb.tile([128, H * 8], F32, tag="prod")
            nc.vector.tensor_mul(out=prod[:], in0=vn[:], in1=cls_all[:])
            sig_col = sb.tile([128, 8], F32, tag="sig_col")
            nc.vector.tensor_reduce(
                out=sig_col[:],
                in_=prod[:].rearrange("p (h j) -> p j h", j=8),
                axis=AX,
                op=OP.add,
            )

            # sequential f32 cumulative sum on a single row
            sig_row = sb.tile([1, T], F32, tag="sig_row")
            nc.sync.dma_start(
                out=sig_row[:].rearrange("a (p j) -> a p j", j=8), in_=sig_col[:]
            )
            cdf_row = sb.tile([1, T], F32, tag="cdf_row")
            nc.vector.tensor_tensor_scan(
                out=cdf_row[:], data0=sig_row[:], data1=sig_row[:],
                initial=0.0, op0=OP.add, op1=OP.bypass,
            )
            denom = sb.tile([1, 1], F32, tag="denom")
            nc.vector.tensor_scalar_add(denom[:], cdf_row[0:1, T - 1:T], 1e-6)

            # unnormalized midpoints cdf[t]+cdf[t+1] (sentinel on the last);
            # denom rides along at [T] so one broadcast covers both
            mids_row = sb.tile([1, T + 1], F32, tag="mids_row")
            nc.vector.tensor_add(
                out=mids_row[0:1, 0:T - 1],
                in0=cdf_row[0:1, 0:T - 1],
                in1=cdf_row[0:1, 1:T],
            )
            nc.vector.memset(mids_row[0:1, T - 1:T], 1e30)
            nc.vector.tensor_copy(out=mids_row[0:1, T:T + 1], in_=denom[:])

            # counts[s] = #{t: mid_t < 2*step_s*denom}, steps on partitions
            mids_bc = sb.tile([128, T + 1], F32, tag="mids_bc")
            nc.gpsimd.partition_broadcast(mids_bc[:], mids_row[:], channels=128)
            th_col = sb.tile([128, 2], F32, tag="th_col")
            nc.vector.tensor_scalar(
                out=th_col[:], in0=steps2c[:], scalar1=mids_bc[:, T:T + 1],
                scalar2=None, op0=OP.mult,
            )
            counts_col = sb.tile([128, 2], F32, tag="counts_col")
            cmp = sb.tile([128, T], F32, tag="cmp")
            for c in range(2):
                nc.vector.tensor_scalar(
                    out=cmp[:], in0=mids_bc[:, 0:T], scalar1=th_col[:, c:c + 1],
                    scalar2=None, op0=OP.is_lt,
                )
                nc.vector.tensor_reduce(
                    out=counts_col[:, c:c + 1], in_=cmp[:], axis=AX, op=OP.add
                )

            # dedup on TensorE with 0/1-integer matmuls (exact in fp32):
            # ids are sorted (counts monotone); mark first occurrences, rank
            # them with a triangular matmul, compact by rank via one-hot
            # matmuls. Rank 0 / duplicates fall out as zero padding, CLS id 0
            # occupies position 0. s = p + 128c on partitions.
            srtp1 = sb.tile([128, 2], F32, tag="srtp1")
            nc.vector.tensor_scalar_add(srtp1[:], counts_col[:], 1.0)
            ps_sh = pp.tile([128, 2], F32, tag="ps_sh")
            nc.tensor.matmul(out=ps_sh[:, 0:1], lhsT=SH, rhs=srtp1[:, 0:1],
                             start=True, stop=True)
            nc.tensor.matmul(out=ps_sh[:, 1:2], lhsT=SH, rhs=srtp1[:, 1:2],
                             start=True, stop=False)
            nc.tensor.matmul(out=ps_sh[:, 1:2], lhsT=E127, rhs=srtp1[:, 0:1],
                             start=False, stop=True)
            first = sb.tile([128, 2], F32, tag="first")
            nc.vector.tensor_tensor(out=first[:], in0=srtp1[:], in1=ps_sh[:],
                                    op=OP.not_equal)
            nc.vector.memset(first[0:1, 0:1], 1.0)
            ps_cum = pp.tile([128, 2], F32, tag="ps_cum")
            nc.tensor.matmul(out=ps_cum[:, 0:1], lhsT=U_, rhs=first[:, 0:1],
                             start=True, stop=True)
            nc.tensor.matmul(out=ps_cum[:, 1:2], lhsT=U_, rhs=first[:, 1:2],
                             start=True, stop=False)
            nc.tensor.matmul(out=ps_cum[:, 1:2], lhsT=AL, rhs=first[:, 0:1],
                             start=False, stop=True)
            # rank = first ? cum : 999 (outside the 0..255 one-hot range)
            rank = sb.tile([128, 2], F32, tag="rank")
            nc.vector.tensor_scalar(out=rank[:], in0=ps_cum[:], scalar1=-999.0,
                                    scalar2=None, op0=OP.add)
            nc.vector.tensor_mul(out=rank[:], in0=rank[:], in1=first[:])
            nc.vector.tensor_scalar(out=rank[:], in0=rank[:], scalar1=999.0,
                                    scalar2=None, op0=OP.add)
            # uniq_ids[j'] = sum_i srtp1[i] * [rank[i] == j']
            ps_u = pp.tile([128, 2], F32, tag="ps_u")
            eq = sb.tile([128, 128], F32, tag="eq")
            for jc in range(2):
                for ic in range(2):
                    nc.vector.tensor_scalar(
                        out=eq[:], in0=IOTA[:, jc * 128:(jc + 1) * 128],
                        scalar1=rank[:, ic:ic + 1], scalar2=None,
                        op0=OP.is_equal,
                    )
                    nc.tensor.matmul(
                        out=ps_u[:, jc:jc + 1], lhsT=eq[:],
                        rhs=srtp1[:, ic:ic + 1],
                        start=(ic == 0), stop=(ic == 1),
                    )
            idx_col = sb.tile([128, 2], I32, tag="idx_col")
            nc.vector.tensor_copy(out=idx_col[:], in_=ps_u[:])
            nc.sync.dma_start(
                out=out_ids[b:b + 1, :].rearrange("a (c p) -> a p c", c=2),
                in_=idx_col[:],
            )

            # gather the selected attention rows per head, stream to output
            for h in range(H):
                for c in range(2):
                    g = gp.tile([128, N], F32, tag="g")
                    nc.gpsimd.indirect_dma_start(
                        out=g[:],
                        out_offset=None,
                        in_=attn_ap[:, :],
                        in_offset=bass.IndirectOffsetOnAxis(
                            ap=idx_col[:, c:c + 1], axis=0
                        ),
                        element_offset=(b * H + h) * N * N,
                    )
                    nc.scalar.dma_start(
                        out=out_attn[b, h, c * 128:(c + 1) * 128, :], in_=g[:]
                    )


def _build_core_graph():
    nc = bacc.Bacc("TRN2", target_bir_lowering=False, debug=False)
    ins = {
        "x": nc.dram_tensor("x", [B_LOC, H, N, D], F32, kind="ExternalInput").ap(),
        "attn": nc.dram_tensor("attn", [ROWS, N], F32, kind="ExternalInput").ap(),
        "steps2c": nc.dram_tensor(
            "steps2c", [128, 2], F32, kind="ExternalInput"
        ).ap(),
        "consts": nc.dram_tensor(
            "consts", [128, 4 * 128 + 256], F32, kind="ExternalInput"
        ).ap(),
    }
    outs = {
        "out_attn": nc.dram_tensor(
            "out_attn", [B_LOC, H, K, N], F32, kind="ExternalOutput"
        ).ap(),
        "out_ids": nc.dram_tensor(
            "out_ids", [B_LOC, K], I32, kind="ExternalOutput"
        ).ap(),
    }
    with tile.TileContext(nc) as tc:
        _emit(tc, ins, outs)
    nc.compile()
    return nc


_NC_CACHE = None


def _get_graph():
    global _NC_CACHE
    if _NC_CACHE is None:
        _NC_CACHE = _build_core_graph()
    return _NC_CACHE


def _steps2c_host():
    steps = ((1.0 + 2.0 * np.arange(S, dtype=np.float32)) / (2.0 * K)).astype(
        np.float32
    )
    s2 = np.empty(256, dtype=np.float32)
    s2[:S] = 2.0 * steps
    s2[S] = 2.0 * steps[-1]  # pad, never used
    return np.ascontiguousarray(s2.reshape(2, 128).T)


def _consts_host():
    p = np.arange(128)
    U_ = (p[:, None] <= p[None, :]).astype(np.float32)
    AL = np.ones((128, 128), np.float32)
    SH = (p[:, None] == p[None, :] - 1).astype(np.float32)
    E127 = np.zeros((128, 128), np.float32)
    E127[127, 0] = 1.0
    IOTA = np.broadcast_to(np.arange(256, dtype=np.float32)[None, :], (128, 256))
    return np.ascontiguousarray(np.concatenate([U_, AL, SH, E127, IOTA], axis=1))


def kernel(x, attn, mask, sample_count, _profile_out=None):
    x = np.asarray(x)
    attn = np.asarray(attn)
    mask = np.asarray(mask)
    sc = int(np.asarray(sample_count))
    assert x.shape == (B, H, N, D) and attn.shape == (B, H, N, N)
    assert sc == K, f"kernel compiled for sample_count={K}, got {sc}"
    assert bool(np.all(mask)), "kernel assumes an all-True mask (spec fill=ones)"

    steps2c = _steps2c_host()
    consts = _consts_host()
    nc = _get_graph()
    in_maps = []
    for c in range(N_CORES):
        bsl = slice(c * B_LOC, (c + 1) * B_LOC)
        in_maps.append(
            {
                "x": np.ascontiguousarray(x[bsl]).astype(np.float32, copy=False),
                "attn": np.ascontiguousarray(attn[bsl]).reshape(ROWS, N),
                "steps2c": steps2c,
                "consts": consts,
            }
        )

    want_trace = _profile_out is not None
    try:
        res = run_bass_kernel_spmd(
            nc, in_maps, core_ids=list(range(N_CORES)), trace=want_trace
        )
    except ImportError:
        res = run_bass_kernel_spmd(
            nc, in_maps, core_ids=list(range(N_CORES)), trace=False
        )
    if want_trace:
        _profile_out["exec_time_ns"] = res.exec_time_ns
        _profile_out["results"] = res

    new_attn = np.concatenate([r["out_attn"] for r in res.results], axis=0)
    uniq_ids = np.concatenate([r["out_ids"] for r in res.results], axis=0).astype(
        np.int32
    )
    new_mask = uniq_ids != 0
    new_mask[:, 0] = True
    return new_attn.astype(np.float32, copy=False), new_mask, uniq_ids


# revision 24
# speedup vs baseline: 1.2769x; 1.2769x over previous
"""AdaptiveTokenSampling on 8 TRN2 NeuronCores (Bass/Tile, data-parallel over batch).

kernel(**inputs) takes the FULL inputs and returns the FULL outputs
(new_attn f32, new_mask bool, uniq_ids int32), sharding batch B=16 as 2
batches per core. All per-batch work (scoring, CDF, inverse-CDF sampling,
dedup, gather) runs on-device; the host only shards inputs and concatenates
shard outputs.

Numerical strategy: the sampled token ids are discrete argmin decisions, so
the on-device CDF must track the float32 reference bit-closely. All
real-valued arithmetic runs on DVE/ACT in IEEE f32 (sequential prefix scan
for the cumsum); sqrt is computed as v*rsqrt(v) with a table seed polished
by two Newton iterations. The nearest-CDF-index argmin is computed as a
midpoint count — sampled[s]-1 = #{t: cdf[t]+cdf[t+1] < 2*step_s*total'} —
which reproduces jnp.argmin's first-index tie-breaking exactly.
"""

from contextlib import ExitStack

import numpy as np

import concourse.bacc as bacc
import concourse.bass as bass
import concourse.mybir as mybir
import concourse.tile as tile
from concourse.bass_utils import run_bass_kernel_spmd

F32 = mybir.dt.float32
I32 = mybir.dt.int32
I16 = mybir.dt.int16

N_CORES = 8
B = 16
B_LOC = B // N_CORES
H = 16
N = 1025
T = 1024
D = 64
K = 256
S = K - 1
ROWS = B_LOC * H * N

AX = mybir.AxisListType.X
OP = mybir.AluOpType
AF = mybir.ActivationFunctionType


def _emit(tc, ins, outs):
    """Emit one NeuronCore's program (B_LOC batches, all heads) into tc."""
    nc = tc.nc
    x_ap = ins["x"]              # (B_LOC,16,1025,64) f32
    attn_ap = ins["attn"]        # (ROWS,1025) f32
    steps2_ap = ins["steps2c"]   # (128,2) f32: 2*steps[p+128*c] (pad tail)
    out_attn = outs["out_attn"]  # (B_LOC,16,256,1025) f32
    out_ids = outs["out_ids"]    # (B_LOC,256) int32

    attn3 = attn_ap.rearrange("(r n) m -> r n m", n=N)

    with ExitStack() as ctx:
        cp = ctx.enter_context(tc.tile_pool(name="const", bufs=1))
        sb = ctx.enter_context(tc.tile_pool(name="sb", bufs=2))
        xp = ctx.enter_context(tc.tile_pool(name="xp", bufs=2))
        gp = ctx.enter_context(tc.tile_pool(name="gp", bufs=8))
        pp = ctx.enter_context(tc.tile_pool(name="pp", bufs=2, space="PSUM"))


        # phase A: batch 0's x halves get both HWDGE rings to themselves
        # (each ring spans 8 SDMA engines); batch 1's x and the cls rows ride
        # the gpsimd SWDGE ring, which is idle until the first gathers. The
        # sequencers are in-order, so bulk loads go ahead of everything.
        # cls rows first on the gpsimd ring (needed by the sig multiply);
        # then batch 0's x over all three DMA rings (fastest possible head)
        # and batch 1's over the two HWDGE rings only, so the gpsimd ring
        # frees up for batch 0's broadcast + gathers.
        cls_l, xh_l = [], []
        for b in range(B_LOC):
            cls_all = sb.tile([128, H * 8], F32, tag="cls_all")
            nc.gpsimd.dma_start(
                out=cls_all[:].rearrange("p (h j) -> p h j", j=8),
                in_=bass.AP(
                    attn_ap.tensor,
                    b * H * N * N + 1,
                    [[8, 128], [N * N, H], [1, 8]],
                ),
            )
            cls_l.append(cls_all)
        for b in range(B_LOC):
            xq = sb.tile([128, H * 512], F32, tag="xq")
            xh_l.append(xq)
            rings = (nc.sync, nc.scalar, nc.gpsimd) if b == 0 else (
                nc.sync, nc.scalar)
            for pc in range(8):
                rings[pc % len(rings)].dma_start(
                    out=xq[:, pc * 1024:(pc + 1) * 1024],
                    in_=x_ap[b, 2 * pc:2 * pc + 2, 1:, :].rearrange(
                        "h (p j) d -> p h (j d)", p=128
                    ),
                )

        steps2c = cp.tile([128, 2], F32)
        nc.sync.dma_start(out=steps2c[:], in_=steps2_ap[:, :])
        # integer/selection constants for the matmul-based dedup
        km = cp.tile([128, 4 * 128 + 256], F32)
        nc.sync.dma_start(out=km[:], in_=ins["consts"][:, :])
        U_ = km[:, 0:128]          # U[p,j] = p<=j
        AL = km[:, 128:256]        # all-ones
        SH = km[:, 256:384]        # SH[p,j] = (p == j-1)
        E127 = km[:, 384:512]      # [p,j] = (p==127 and j==0)
        IOTA = km[:, 512:768]      # IOTA[p,f] = f

        for b in range(B_LOC):
            cls_all = cls_l[b]
            xq = xh_l[b]

            # squared value norms: ACT squares + DVE grouped-reduces,
            # pipelined at 2-head granularity behind the split loads
            vnsq = sb.tile([128, H * 8], F32, tag="vnsq")
            for pc in range(8):
                sq = xp.tile([128, 2 * 512], F32, tag="sq")
                nc.scalar.activation(
                    out=sq[:], in_=xq[:, pc * 1024:(pc + 1) * 1024],
                    func=AF.Square,
                )
                nc.vector.tensor_reduce(
                    out=vnsq[:, pc * 16:(pc + 1) * 16],
                    in_=sq[:].rearrange("p (hj d) -> p hj d", d=D),
                    axis=AX,
                    op=OP.add,
                )

            # vn = vnsq * rsqrt(vnsq); seed 1/ACT-sqrt + 2 Newton iterations
            r = sb.tile([128, H * 8], F32, tag="r")
            nc.scalar.activation(out=r[:], in_=vnsq[:], func=AF.Sqrt)
            nc.vector.reciprocal(out=r[:], in_=r[:])
            t1 = sb.tile([128, H * 8], F32, tag="t1")
            for _ in range(2):
                nc.vector.tensor_mul(out=t1[:], in0=r[:], in1=r[:])
                nc.vector.tensor_mul(out=t1[:], in0=t1[:], in1=vnsq[:])
                nc.vector.tensor_scalar(
                    out=t1[:], in0=t1[:], scalar1=-0.5, scalar2=1.5,
                    op0=OP.mult, op1=OP.add,
                )
                nc.vector.tensor_mul(out=r[:], in0=r[:], in1=t1[:])
            vn = sb.tile([128, H * 8], F32, tag="vn")
            nc.vector.tensor_mul(out=vn[:], in0=vnsq[:], in1=r[:])

            # head-summed significance
            prod = sb.tile([128, H * 8], F32, tag="prod")
            nc.vector.tensor_mul(out=prod[:], in0=vn[:], in1=cls_all[:])
            sig_col = sb.tile([128, 8], F32, tag="sig_col")
            nc.vector.tensor_reduce(
                out=sig_col[:],
                in_=prod[:].rearrange("p (h j) -> p j h", j=8),
                axis=AX,
                op=OP.add,
            )

            # sequential f32 cumulative sum on a single row
            sig_row = sb.tile([1, T], F32, tag="sig_row")
            nc.sync.dma_start(
                out=sig_row[:].rearrange("a (p j) -> a p j", j=8), in_=sig_col[:]
            )
            cdf_row = sb.tile([1, T], F32, tag="cdf_row")
            nc.vector.tensor_tensor_scan(
                out=cdf_row[:], data0=sig_row[:], data1=sig_row[:],
                initial=0.0, op0=OP.add, op1=OP.bypass,
            )
            denom = sb.tile([1, 1], F32, tag="denom")
            nc.vector.tensor_scalar_add(denom[:], cdf_row[0:1, T - 1:T], 1e-6)

            # unnormalized midpoints cdf[t]+cdf[t+1] (sentinel on the last);
            # denom rides along at [T] so one broadcast covers both
            mids_row = sb.tile([1, T + 1], F32, tag="mids_row")
            nc.vector.tensor_add(
                out=mids_row[0:1, 0:T - 1],
                in0=cdf_row[0:1, 0:T - 1],
                in1=cdf_row[0:1, 1:T],
            )
            nc.vector.memset(mids_row[0:1, T - 1:T], 1e30)
            nc.vector.tensor_copy(out=mids_row[0:1, T:T + 1], in_=denom[:])

            # counts[s] = #{t: mid_t < 2*step_s*denom}, steps on partitions
            mids_bc = sb.tile([128, T + 1], F32, tag="mids_bc")
            nc.gpsimd.partition_broadcast(mids_bc[:], mids_row[:], channels=128)
            th_col = sb.tile([128, 2], F32, tag="th_col")
            nc.vector.tensor_scalar(
                out=th_col[:], in0=steps2c[:], scalar1=mids_bc[:, T:T + 1],
                scalar2=None, op0=OP.mult,
            )
            counts_col = sb.tile([128, 2], F32, tag="counts_col")
            cmp = sb.tile([128, T], F32, tag="cmp")
            for c in range(2):
                nc.vector.tensor_scalar(
                    out=cmp[:], in0=mids_bc[:, 0:T], scalar1=th_col[:, c:c + 1],
                    scalar2=None, op0=OP.is_lt,
                )
                nc.vector.tensor_reduce(
                    out=counts_col[:, c:c + 1], in_=cmp[:], axis=AX, op=OP.add
                )

            # dedup on TensorE with 0/1-integer matmuls (exact in fp32):
            # ids are sorted (counts monotone); mark first occurrences, rank
            # them with a triangular matmul, compact by rank via one-hot
            # matmuls. Rank 0 / duplicates fall out as zero padding, CLS id 0
            # occupies position 0. s = p + 128c on partitions.
            srtp1 = sb.tile([128, 2], F32, tag="srtp1")
            nc.vector.tensor_scalar_add(srtp1[:], counts_col[:], 1.0)
            ps_sh = pp.tile([128, 2], F32, tag="ps_sh")
            nc.tensor.matmul(out=ps_sh[:, 0:1], lhsT=SH, rhs=srtp1[:, 0:1],
                             start=True, stop=True)
            nc.tensor.matmul(out=ps_sh[:, 1:2], lhsT=SH, rhs=srtp1[:, 1:2],
                             start=True, stop=False)
            nc.tensor.matmul(out=ps_sh[:, 1:2], lhsT=E127, rhs=srtp1[:, 0:1],
                             start=False, stop=True)
            first = sb.tile([128, 2], F32, tag="first")
            nc.vector.tensor_tensor(out=first[:], in0=srtp1[:], in1=ps_sh[:],
                                    op=OP.not_equal)
            nc.vector.memset(first[0:1, 0:1], 1.0)
            ps_cum = pp.tile([128, 2], F32, tag="ps_cum")
            nc.tensor.matmul(out=ps_cum[:, 0:1], lhsT=U_, rhs=first[:, 0:1],
                             start=True, stop=True)
            nc.tensor.matmul(out=ps_cum[:, 1:2], lhsT=U_, rhs=first[:, 1:2],
                             start=True, stop=False)
            nc.tensor.matmul(out=ps_cum[:, 1:2], lhsT=AL, rhs=first[:, 0:1],
                             start=False, stop=True)
            # rank = first ? cum : 999 (outside the 0..255 one-hot range)
            rank = sb.tile([128, 2], F32, tag="rank")
            nc.vector.tensor_scalar(out=rank[:], in0=ps_cum[:], scalar1=-999.0,
                                    scalar2=None, op0=OP.add)
            nc.vector.tensor_mul(out=rank[:], in0=rank[:], in1=first[:])
            nc.vector.tensor_scalar(out=rank[:], in0=rank[:], scalar1=999.0,
                                    scalar2=None, op0=OP.add)
            # uniq_ids[j'] = sum_i srtp1[i] * [rank[i] == j']
            ps_u = pp.tile([128, 2], F32, tag="ps_u")
            eq = sb.tile([128, 128], F32, tag="eq")
            for jc in range(2):
                for ic in range(2):
                    nc.vector.tensor_scalar(
                        out=eq[:], in0=IOTA[:, jc * 128:(jc + 1) * 128],
                        scalar1=rank[:, ic:ic + 1], scalar2=None,
                        op0=OP.is_equal,
                    )
                    nc.tensor.matmul(
                        out=ps_u[:, jc:jc + 1], lhsT=eq[:],
                        rhs=srtp1[:, ic:ic + 1],
                        start=(ic == 0), stop=(ic == 1),
                    )
            idx_col = sb.tile([128, 2], I32, tag="idx_col")
            nc.vector.tensor_copy(out=idx_col[:], in_=ps_u[:])
            nc.sync.dma_start(
                out=out_ids[b:b + 1, :].rearrange("a (c p) -> a p c", c=2),
                in_=idx_col[:],
            )

            # gather the selected attention rows per head, stream to output
            # (one index per partition: the HW indirect DMA ignores a second
            # free-axis index even though the simulator accepts it)
            for h in range(H):
                for c in range(2):
                    g = gp.tile([128, N], F32, tag="g")
                    nc.gpsimd.indirect_dma_start(
                        out=g[:],
                        out_offset=None,
                        in_=attn_ap[:, :],
                        in_offset=bass.IndirectOffsetOnAxis(
                            ap=idx_col[:, c:c + 1], axis=0
                        ),
                        element_offset=(b * H + h) * N * N,
                    )
                    nc.scalar.dma_start(
                        out=out_attn[b, h, c * 128:(c + 1) * 128, :], in_=g[:]
                    )


def _build_core_graph():
    nc = bacc.Bacc("TRN2", target_bir_lowering=False, debug=False)
    ins = {
        "x": nc.dram_tensor("x", [B_LOC, H, N, D], F32, kind="ExternalInput").ap(),
        "attn": nc.dram_tensor("attn", [ROWS, N], F32, kind="ExternalInput").ap(),
        "steps2c": nc.dram_tensor(
            "steps2c", [128, 2], F32, kind="ExternalInput"
        ).ap(),
        "consts": nc.dram_tensor(
            "consts", [128, 4 * 128 + 256], F32, kind="ExternalInput"
        ).ap(),
    }
    outs = {
        "out_attn": nc.dram_tensor(
            "out_attn", [B_LOC, H, K, N], F32, kind="ExternalOutput"
        ).ap(),
        "out_ids": nc.dram_tensor(
            "out_ids", [B_LOC, K], I32, kind="ExternalOutput"
        ).ap(),
    }
    with tile.TileContext(nc) as tc:
        _emit(tc, ins, outs)
    nc.compile()
    return nc


_NC_CACHE = None


def _get_graph():
    global _NC_CACHE
    if _NC_CACHE is None:
        _NC_CACHE = _build_core_graph()
    return _NC_CACHE


def _steps2c_host():
    steps = ((1.0 + 2.0 * np.arange(S, dtype=np.float32)) / (2.0 * K)).astype(
        np.float32
    )
    s2 = np.empty(256, dtype=np.float32)
    s2[:S] = 2.0 * steps
    s2[S] = 2.0 * steps[-1]  # pad, never used
    return np.ascontiguousarray(s2.reshape(2, 128).T)


def _consts_host():
    p = np.arange(128)
    U_ = (p[:, None] <= p[None, :]).astype(np.float32)
    AL = np.ones((128, 128), np.float32)
    SH = (p[:, None] == p[None, :] - 1).astype(np.float32)
    E127 = np.zeros((128, 128), np.float32)
    E127[127, 0] = 1.0
    IOTA = np.broadcast_to(np.arange(256, dtype=np.float32)[None, :], (128, 256))
    return np.ascontiguousarray(np.concatenate([U_, AL, SH, E127, IOTA], axis=1))


def kernel(x, attn, mask, sample_count, _profile_out=None):
    x = np.asarray(x)
    attn = np.asarray(attn)
    mask = np.asarray(mask)
    sc = int(np.asarray(sample_count))
    assert x.shape == (B, H, N, D) and attn.shape == (B, H, N, N)
    assert sc == K, f"kernel compiled for sample_count={K}, got {sc}"
    assert bool(np.all(mask)), "kernel assumes an all-True mask (spec fill=ones)"

    steps2c = _steps2c_host()
    consts = _consts_host()
    nc = _get_graph()
    in_maps = []
    for c in range(N_CORES):
        bsl = slice(c * B_LOC, (c + 1) * B_LOC)
        in_maps.append(
            {
                "x": np.ascontiguousarray(x[bsl]).astype(np.float32, copy=False),
                "attn": np.ascontiguousarray(attn[bsl]).reshape(ROWS, N),
                "steps2c": steps2c,
                "consts": consts,
            }
        )

    want_trace = _profile_out is not None
    try:
        res = run_bass_kernel_spmd(
            nc, in_maps, core_ids=list(range(N_CORES)), trace=want_trace
        )
    except ImportError:
        res = run_bass_kernel_spmd(
            nc, in_maps, core_ids=list(range(N_CORES)), trace=False
        )
    if want_trace:
        _profile_out["exec_time_ns"] = res.exec_time_ns
        _profile_out["results"] = res

    new_attn = np.concatenate([r["out_attn"] for r in res.results], axis=0)
    uniq_ids = np.concatenate([r["out_ids"] for r in res.results], axis=0).astype(
        np.int32
    )
    new_mask = uniq_ids != 0
    new_mask[:, 0] = True
    return new_attn.astype(np.float32, copy=False), new_mask, uniq_ids


# revision 25
# speedup vs baseline: 1.3134x; 1.0286x over previous
"""AdaptiveTokenSampling on 8 TRN2 NeuronCores (Bass/Tile, data-parallel over batch).

kernel(**inputs) takes the FULL inputs and returns the FULL outputs
(new_attn f32, new_mask bool, uniq_ids int32), sharding batch B=16 as 2
batches per core. All per-batch work (scoring, CDF, inverse-CDF sampling,
dedup, gather) runs on-device; the host only shards inputs and concatenates
shard outputs.

Numerical strategy: the sampled token ids are discrete argmin decisions, so
the on-device CDF must track the float32 reference bit-closely. All
real-valued arithmetic runs on DVE/ACT in IEEE f32 (sequential prefix scan
for the cumsum); sqrt is computed as v*rsqrt(v) with a table seed polished
by two Newton iterations. The nearest-CDF-index argmin is computed as a
midpoint count — sampled[s]-1 = #{t: cdf[t]+cdf[t+1] < 2*step_s*total'} —
which reproduces jnp.argmin's first-index tie-breaking exactly.
"""

from contextlib import ExitStack

import numpy as np

import concourse.bacc as bacc
import concourse.bass as bass
import concourse.mybir as mybir
import concourse.tile as tile
from concourse.bass_utils import run_bass_kernel_spmd

F32 = mybir.dt.float32
I32 = mybir.dt.int32
I16 = mybir.dt.int16

N_CORES = 8
B = 16
B_LOC = B // N_CORES
H = 16
N = 1025
T = 1024
D = 64
K = 256
S = K - 1
ROWS = B_LOC * H * N

AX = mybir.AxisListType.X
OP = mybir.AluOpType
AF = mybir.ActivationFunctionType


def _emit(tc, ins, outs):
    """Emit one NeuronCore's program (B_LOC batches, all heads) into tc."""
    nc = tc.nc
    x_ap = ins["x"]              # (B_LOC,16,1025,64) f32
    attn_ap = ins["attn"]        # (ROWS,1025) f32
    steps2_ap = ins["steps2c"]   # (128,2) f32: 2*steps[p+128*c] (pad tail)
    out_attn = outs["out_attn"]  # (B_LOC,16,256,1025) f32
    out_ids = outs["out_ids"]    # (B_LOC,256) int32

    attn3 = attn_ap.rearrange("(r n) m -> r n m", n=N)

    with ExitStack() as ctx:
        cp = ctx.enter_context(tc.tile_pool(name="const", bufs=1))
        sb = ctx.enter_context(tc.tile_pool(name="sb", bufs=2))
        xp = ctx.enter_context(tc.tile_pool(name="xp", bufs=2))
        gp = ctx.enter_context(tc.tile_pool(name="gp", bufs=8))
        pp = ctx.enter_context(tc.tile_pool(name="pp", bufs=2, space="PSUM"))


        # phase A: batch 0's x halves get both HWDGE rings to themselves
        # (each ring spans 8 SDMA engines); batch 1's x and the cls rows ride
        # the gpsimd SWDGE ring, which is idle until the first gathers. The
        # sequencers are in-order, so bulk loads go ahead of everything.
        # cls rows first on the gpsimd ring (needed by the sig multiply);
        # then batch 0's x over all three DMA rings (fastest possible head)
        # and batch 1's over the two HWDGE rings only, so the gpsimd ring
        # frees up for batch 0's broadcast + gathers.
        cls_l, xh_l = [], []
        for b in range(B_LOC):
            cls_all = sb.tile([128, H * 8], F32, tag="cls_all")
            nc.gpsimd.dma_start(
                out=cls_all[:].rearrange("p (h j) -> p h j", j=8),
                in_=bass.AP(
                    attn_ap.tensor,
                    b * H * N * N + 1,
                    [[8, 128], [N * N, H], [1, 8]],
                ),
            )
            cls_l.append(cls_all)
        for b in range(B_LOC):
            xq = sb.tile([128, H * 512], F32, tag="xq")
            xh_l.append(xq)
            rings = (nc.sync, nc.scalar, nc.gpsimd) if b == 0 else (
                nc.sync, nc.scalar)
            for pc in range(8):
                rings[pc % len(rings)].dma_start(
                    out=xq[:, pc * 1024:(pc + 1) * 1024],
                    in_=x_ap[b, 2 * pc:2 * pc + 2, 1:, :].rearrange(
                        "h (p j) d -> p h (j d)", p=128
                    ),
                )

        steps2c = cp.tile([128, 2], F32)
        nc.sync.dma_start(out=steps2c[:], in_=steps2_ap[:, :])
        # prewarm the sqrt_and_friends ACT table (covers Square+Sqrt) so the
        # ~2.6us table DMA overlaps the x loads instead of the first square
        warm = cp.tile([1, 1], F32)
        nc.vector.memset(warm[:], 1.0)
        nc.scalar.activation(out=warm[:], in_=warm[:], func=AF.Sqrt)
        # integer/selection constants for the matmul-based dedup
        km = cp.tile([128, 4 * 128 + 256], F32)
        nc.sync.dma_start(out=km[:], in_=ins["consts"][:, :])
        U_ = km[:, 0:128]          # U[p,j] = p<=j
        AL = km[:, 128:256]        # all-ones
        SH = km[:, 256:384]        # SH[p,j] = (p == j-1)
        E127 = km[:, 384:512]      # [p,j] = (p==127 and j==0)
        IOTA = km[:, 512:768]      # IOTA[p,f] = f

        for b in range(B_LOC):
            cls_all = cls_l[b]
            xq = xh_l[b]

            # squared value norms: ACT squares + DVE grouped-reduces,
            # pipelined at 2-head granularity behind the split loads
            vnsq = sb.tile([128, H * 8], F32, tag="vnsq")
            for pc in range(8):
                sq = xp.tile([128, 2 * 512], F32, tag="sq")
                nc.scalar.activation(
                    out=sq[:], in_=xq[:, pc * 1024:(pc + 1) * 1024],
                    func=AF.Square,
                )
                nc.vector.tensor_reduce(
                    out=vnsq[:, pc * 16:(pc + 1) * 16],
                    in_=sq[:].rearrange("p (hj d) -> p hj d", d=D),
                    axis=AX,
                    op=OP.add,
                )

            # vn = vnsq * rsqrt(vnsq); seed 1/ACT-sqrt + 2 Newton iterations
            r = sb.tile([128, H * 8], F32, tag="r")
            nc.scalar.activation(out=r[:], in_=vnsq[:], func=AF.Sqrt)
            nc.vector.reciprocal(out=r[:], in_=r[:])
            t1 = sb.tile([128, H * 8], F32, tag="t1")
            for _ in range(2):
                nc.vector.tensor_mul(out=t1[:], in0=r[:], in1=r[:])
                nc.vector.tensor_mul(out=t1[:], in0=t1[:], in1=vnsq[:])
                nc.vector.tensor_scalar(
                    out=t1[:], in0=t1[:], scalar1=-0.5, scalar2=1.5,
                    op0=OP.mult, op1=OP.add,
                )
                nc.vector.tensor_mul(out=r[:], in0=r[:], in1=t1[:])
            vn = sb.tile([128, H * 8], F32, tag="vn")
            nc.vector.tensor_mul(out=vn[:], in0=vnsq[:], in1=r[:])

            # head-summed significance
            prod = sb.tile([128, H * 8], F32, tag="prod")
            nc.vector.tensor_mul(out=prod[:], in0=vn[:], in1=cls_all[:])
            sig_col = sb.tile([128, 8], F32, tag="sig_col")
            nc.vector.tensor_reduce(
                out=sig_col[:],
                in_=prod[:].rearrange("p (h j) -> p j h", j=8),
                axis=AX,
                op=OP.add,
            )

            # sequential f32 cumulative sum on a single row
            sig_row = sb.tile([1, T], F32, tag="sig_row")
            nc.sync.dma_start(
                out=sig_row[:].rearrange("a (p j) -> a p j", j=8), in_=sig_col[:]
            )
            cdf_row = sb.tile([1, T], F32, tag="cdf_row")
            nc.vector.tensor_tensor_scan(
                out=cdf_row[:], data0=sig_row[:], data1=sig_row[:],
                initial=0.0, op0=OP.add, op1=OP.bypass,
            )
            denom = sb.tile([1, 1], F32, tag="denom")
            nc.vector.tensor_scalar_add(denom[:], cdf_row[0:1, T - 1:T], 1e-6)

            # unnormalized midpoints cdf[t]+cdf[t+1] (sentinel on the last);
            # denom rides along at [T] so one broadcast covers both
            mids_row = sb.tile([1, T + 1], F32, tag="mids_row")
            nc.vector.tensor_add(
                out=mids_row[0:1, 0:T - 1],
                in0=cdf_row[0:1, 0:T - 1],
                in1=cdf_row[0:1, 1:T],
            )
            nc.vector.memset(mids_row[0:1, T - 1:T], 1e30)
            nc.vector.tensor_copy(out=mids_row[0:1, T:T + 1], in_=denom[:])

            # counts[s] = #{t: mid_t < 2*step_s*denom}, steps on partitions
            mids_bc = sb.tile([128, T + 1], F32, tag="mids_bc")
            nc.gpsimd.partition_broadcast(mids_bc[:], mids_row[:], channels=128)
            th_col = sb.tile([128, 2], F32, tag="th_col")
            nc.vector.tensor_scalar(
                out=th_col[:], in0=steps2c[:], scalar1=mids_bc[:, T:T + 1],
                scalar2=None, op0=OP.mult,
            )
            counts_col = sb.tile([128, 2], F32, tag="counts_col")
            cmp = sb.tile([128, T], F32, tag="cmp")
            for c in range(2):
                nc.vector.tensor_scalar(
                    out=cmp[:], in0=mids_bc[:, 0:T], scalar1=th_col[:, c:c + 1],
                    scalar2=None, op0=OP.is_lt,
                )
                nc.vector.tensor_reduce(
                    out=counts_col[:, c:c + 1], in_=cmp[:], axis=AX, op=OP.add
                )

            # dedup on TensorE with 0/1-integer matmuls (exact in fp32):
            # ids are sorted (counts monotone); mark first occurrences, rank
            # them with a triangular matmul, compact by rank via one-hot
            # matmuls. Rank 0 / duplicates fall out as zero padding, CLS id 0
            # occupies position 0. s = p + 128c on partitions.
            srtp1 = sb.tile([128, 2], F32, tag="srtp1")
            nc.vector.tensor_scalar_add(srtp1[:], counts_col[:], 1.0)
            ps_sh = pp.tile([128, 2], F32, tag="ps_sh")
            nc.tensor.matmul(out=ps_sh[:, 0:1], lhsT=SH, rhs=srtp1[:, 0:1],
                             start=True, stop=True)
            nc.tensor.matmul(out=ps_sh[:, 1:2], lhsT=SH, rhs=srtp1[:, 1:2],
                             start=True, stop=False)
            nc.tensor.matmul(out=ps_sh[:, 1:2], lhsT=E127, rhs=srtp1[:, 0:1],
                             start=False, stop=True)
            first = sb.tile([128, 2], F32, tag="first")
            nc.vector.tensor_tensor(out=first[:], in0=srtp1[:], in1=ps_sh[:],
                                    op=OP.not_equal)
            nc.vector.memset(first[0:1, 0:1], 1.0)
            ps_cum = pp.tile([128, 2], F32, tag="ps_cum")
            nc.tensor.matmul(out=ps_cum[:, 0:1], lhsT=U_, rhs=first[:, 0:1],
                             start=True, stop=True)
            nc.tensor.matmul(out=ps_cum[:, 1:2], lhsT=U_, rhs=first[:, 1:2],
                             start=True, stop=False)
            nc.tensor.matmul(out=ps_cum[:, 1:2], lhsT=AL, rhs=first[:, 0:1],
                             start=False, stop=True)
            # rank = first ? cum : 999 (outside the 0..255 one-hot range)
            rank = sb.tile([128, 2], F32, tag="rank")
            nc.vector.tensor_scalar(out=rank[:], in0=ps_cum[:], scalar1=-999.0,
                                    scalar2=None, op0=OP.add)
            nc.vector.tensor_mul(out=rank[:], in0=rank[:], in1=first[:])
            nc.vector.tensor_scalar(out=rank[:], in0=rank[:], scalar1=999.0,
                                    scalar2=None, op0=OP.add)
            # uniq_ids[j'] = sum_i srtp1[i] * [rank[i] == j']
            ps_u = pp.tile([128, 2], F32, tag="ps_u")
            eq = sb.tile([128, 4 * 128], F32, tag="eq")
            for jc in range(2):
                for ic in range(2):
                    nc.vector.tensor_scalar(
                        out=eq[:, (2 * jc + ic) * 128:(2 * jc + ic + 1) * 128],
                        in0=IOTA[:, jc * 128:(jc + 1) * 128],
                        scalar1=rank[:, ic:ic + 1], scalar2=None,
                        op0=OP.is_equal,
                    )
            for jc in range(2):
                for ic in range(2):
                    nc.tensor.matmul(
                        out=ps_u[:, jc:jc + 1],
                        lhsT=eq[:, (2 * jc + ic) * 128:(2 * jc + ic + 1) * 128],
                        rhs=srtp1[:, ic:ic + 1],
                        start=(ic == 0), stop=(ic == 1),
                    )
            idx_col = sb.tile([128, 2], I32, tag="idx_col")
            nc.vector.tensor_copy(out=idx_col[:], in_=ps_u[:])
            nc.sync.dma_start(
                out=out_ids[b:b + 1, :].rearrange("a (c p) -> a p c", c=2),
                in_=idx_col[:],
            )

            # gather the selected attention rows per head, stream to output
            # (one index per partition: the HW indirect DMA ignores a second
            # free-axis index even though the simulator accepts it)
            for h in range(H):
                for c in range(2):
                    g = gp.tile([128, N], F32, tag="g")
                    nc.gpsimd.indirect_dma_start(
                        out=g[:],
                        out_offset=None,
                        in_=attn_ap[:, :],
                        in_offset=bass.IndirectOffsetOnAxis(
                            ap=idx_col[:, c:c + 1], axis=0
                        ),
                        element_offset=(b * H + h) * N * N,
                    )
                    nc.scalar.dma_start(
                        out=out_attn[b, h, c * 128:(c + 1) * 128, :], in_=g[:]
                    )


def _build_core_graph():
    nc = bacc.Bacc("TRN2", target_bir_lowering=False, debug=False)
    ins = {
        "x": nc.dram_tensor("x", [B_LOC, H, N, D], F32, kind="ExternalInput").ap(),
        "attn": nc.dram_tensor("attn", [ROWS, N], F32, kind="ExternalInput").ap(),
        "steps2c": nc.dram_tensor(
            "steps2c", [128, 2], F32, kind="ExternalInput"
        ).ap(),
        "consts": nc.dram_tensor(
            "consts", [128, 4 * 128 + 256], F32, kind="ExternalInput"
        ).ap(),
    }
    outs = {
        "out_attn": nc.dram_tensor(
            "out_attn", [B_LOC, H, K, N], F32, kind="ExternalOutput"
        ).ap(),
        "out_ids": nc.dram_tensor(
            "out_ids", [B_LOC, K], I32, kind="ExternalOutput"
        ).ap(),
    }
    with tile.TileContext(nc) as tc:
        _emit(tc, ins, outs)
    nc.compile()
    return nc


_NC_CACHE = None


def _get_graph():
    global _NC_CACHE
    if _NC_CACHE is None:
        _NC_CACHE = _build_core_graph()
    return _NC_CACHE


def _steps2c_host():
    steps = ((1.0 + 2.0 * np.arange(S, dtype=np.float32)) / (2.0 * K)).astype(
        np.float32
    )
    s2 = np.empty(256, dtype=np.float32)
    s2[:S] = 2.0 * steps
    s2[S] = 2.0 * steps[-1]  # pad, never used
    return np.ascontiguousarray(s2.reshape(2, 128).T)


def _consts_host():
    p = np.arange(128)
    U_ = (p[:, None] <= p[None, :]).astype(np.float32)
    AL = np.ones((128, 128), np.float32)
    SH = (p[:, None] == p[None, :] - 1).astype(np.float32)
    E127 = np.zeros((128, 128), np.float32)
    E127[127, 0] = 1.0
    IOTA = np.broadcast_to(np.arange(256, dtype=np.float32)[None, :], (128, 256))
    return np.ascontiguousarray(np.concatenate([U_, AL, SH, E127, IOTA], axis=1))


def kernel(x, attn, mask, sample_count, _profile_out=None):
    x = np.asarray(x)
    attn = np.asarray(attn)
    mask = np.asarray(mask)
    sc = int(np.asarray(sample_count))
    assert x.shape == (B, H, N, D) and attn.shape == (B, H, N, N)
    assert sc == K, f"kernel compiled for sample_count={K}, got {sc}"
    assert bool(np.all(mask)), "kernel assumes an all-True mask (spec fill=ones)"

    steps2c = _steps2c_host()
    consts = _consts_host()
    nc = _get_graph()
    in_maps = []
    for c in range(N_CORES):
        bsl = slice(c * B_LOC, (c + 1) * B_LOC)
        in_maps.append(
            {
                "x": np.ascontiguousarray(x[bsl]).astype(np.float32, copy=False),
                "attn": np.ascontiguousarray(attn[bsl]).reshape(ROWS, N),
                "steps2c": steps2c,
                "consts": consts,
            }
        )

    want_trace = _profile_out is not None
    try:
        res = run_bass_kernel_spmd(
            nc, in_maps, core_ids=list(range(N_CORES)), trace=want_trace
        )
    except ImportError:
        res = run_bass_kernel_spmd(
            nc, in_maps, core_ids=list(range(N_CORES)), trace=False
        )
    if want_trace:
        _profile_out["exec_time_ns"] = res.exec_time_ns
        _profile_out["results"] = res

    new_attn = np.concatenate([r["out_attn"] for r in res.results], axis=0)
    uniq_ids = np.concatenate([r["out_ids"] for r in res.results], axis=0).astype(
        np.int32
    )
    new_mask = uniq_ids != 0
    new_mask[:, 0] = True
    return new_attn.astype(np.float32, copy=False), new_mask, uniq_ids
